# revision 8
# baseline (speedup 1.0000x reference)
"""CompiledLogicNet forward on 8 Trainium2 NeuronCores.

Strategy (pure data parallelism, per the sharding hint):
- Shard the batch (2048) across 8 cores: 256 samples/core.
- Bit-pack the per-core batch: 256 samples -> 8 int32 words per gate.
- Layer 0: threshold x>0.5 and pack bits via a DVE shift/or tree.
- Gate layers: per-gate operand gathers done with chunked dma_gather
  (32B payload rows, 256B-stride padded h in DRAM), gate op evaluated
  uniformly in ANF form r = D0 ^ (D1&A) ^ (D2&B) ^ (D3&(A&B)) with
  per-gate constant masks derived from the op ids on the host.
- GroupSum: nibble-spread + segmented reduce + ones-matmul over
  partitions.

The Bass program is JIT-built inside kernel(); gate indices/ops are
baked into uploaded constant tensors (not the program), x stays a
device input.
"""
import sys

for _p in ("/opt/trn_rl_repo", "/root/.axon_site/_ro/trn_rl_repo"):
    if _p not in sys.path:
        sys.path.insert(0, _p)

import numpy as np
import concourse.bass as bass
import concourse.bacc as bacc
import concourse.mybir as mybir
from concourse import ap_utils
from concourse.bass_primitives import MemorySpace
from concourse.tile import TileContext
from concourse.bass_utils import run_bass_kernel_spmd

B, NI, W, NC = 2048, 1024, 16000, 10
CORES = 8
BS = B // CORES          # samples per core
NW = BS // 32            # packed words per gate (8)
STRIDE = 64              # DRAM row stride in int32 (256B) for dma_gather
P = 128
S = W // P               # 125 slots for layers 1..3
CH = 1024                # gather chunk size (ring-safe)
SLOTC = 13               # final layer: slots per class (13*128=1664 >= 1600)
S4 = SLOTC * NC          # 130 final slots
W4 = S4 * P              # 16640 padded final gates
AND, OR, XOR, SHL, SHR = (mybir.AluOpType.bitwise_and, mybir.AluOpType.bitwise_or,
                          mybir.AluOpType.bitwise_xor, mybir.AluOpType.logical_shift_left,
                          mybir.AluOpType.logical_shift_right)
I32 = mybir.dt.int32


def _dma_gather_small(eng, out_ap, in_ap, idxs_ap, num_idxs, elem_size, elem_step,
                      queue_num=0):
    """bass.dma_gather minus the elem_size%256 assert (non-transpose, DRAM src).

    Row *stride* (elem_step*4B) must still be a multiple of 256B; the payload
    (elem_size) is free — confirmed against the Q7 ucode and on HW.
    """
    assert idxs_ap.dtype == mybir.dt.int16
    assert in_ap.dtype == out_ap.dtype
    assert in_ap.space == MemorySpace.DRAM
    assert ap_utils.ap_is_contiguous(out_ap.ap[1:])
    assert ap_utils.ap_is_contiguous(idxs_ap.ap[1:])
    assert in_ap.ap[0][0] == elem_step
    assert in_ap.ap[-1][1] == out_ap.ap[-1][1] == elem_size
    assert out_ap.ap[0][1] * out_ap.ap[1][1] == ((num_idxs + 127) // 128) * 128
    stride_bytes = elem_step * mybir.dt.size(in_ap.dtype)
    assert stride_bytes % 256 == 0 and stride_bytes // 256 < 256

    _in_ap = eng.lower_ap_dma(in_ap, for_custom_bir_dma=True)
    return eng.add_instruction(
        mybir.InstDMAGatherAnt(
            name=eng.bass.get_next_instruction_name(),
            ins=[*_in_ap, eng.lower_ap(idxs_ap),
                 eng.lower_val_access(eng.to_reg(num_idxs))],
            outs=[eng.lower_ap(out_ap)],
            transpose=False,
            num_idxs=num_idxs,
            elem_size=elem_size,
            stride_bytes_256=stride_bytes // 256,
            gen_mode=0,
            single_packet=True,
            queue_num=queue_num,
            sbuf_tokens_per_rank=0,
            sbuf_free_dim_per_rank=0,
            sbuf_free_dim_pad_per_rank=0,
            sbuf_byte_offset=0,
        )
    )


def _wrap_idx_chunked(idx, chunk=CH):
    """[N] -> [128, N/16] int16: per-chunk 16-partition wrap, replicated x8."""
    cols = []
    for c0 in range(0, len(idx), chunk):
        part = np.asarray(idx[c0:c0 + chunk])
        cols.append(part.reshape(len(part) // 16, 16).T.astype(np.int16))
    w16 = np.concatenate(cols, axis=1)          # [16, N/16]
    return np.tile(w16, (CORES, 1))             # [128, N/16]


def _anf_masks(ops):
    """Per-gate ANF coefficient masks (int32 0 / -1) from 4-bit op ids.

    r = d0 ^ (d1&A) ^ (d2&B) ^ (d3&A&B) with op bit3=(0,0), bit2=(0,1),
    bit1=(1,0), bit0=(1,1) per the reference LUT convention.
    """
    t3, t2, t1, t0 = (ops >> 3) & 1, (ops >> 2) & 1, (ops >> 1) & 1, ops & 1
    d0 = t3
    d1 = t3 ^ t1
    d2 = t3 ^ t2
    d3 = t3 ^ t2 ^ t1 ^ t0
    return [(-m).astype(np.int32) for m in (d0, d1, d2, d3)]


def _tile_layout(vec, slots):
    """[slots*128] values -> [128, slots] tile where gate j=(s*128+p) -> (p, s)."""
    return np.asarray(vec).reshape(slots, P).T.copy()


def _chunks(n):
    out = []
    c0 = 0
    while c0 < n:
        out.append((c0, min(CH, n - c0)))
        c0 += CH
    return out


def _build_program():
    nc = bacc.Bacc("TRN2", target_bir_lowering=False, debug=False,
                   num_devices=CORES, num_swdge_queues=4)

    xT = nc.dram_tensor("xT", [NI, BS], mybir.dt.float32, kind="ExternalInput")
    out_y = nc.dram_tensor("out_y", [1, NC * BS], I32, kind="ExternalOutput")

    idx_in, d_in = {}, {}
    for l in range(4):
        n = W if l < 3 else W4
        for ab in "ab":
            idx_in[(l, ab)] = nc.dram_tensor(
                f"idx{ab}{l}", [P, n // 16], mybir.dt.int16, kind="ExternalInput")
        slots = S if l < 3 else S4
        for k in range(4):
            d_in[(l, k)] = nc.dram_tensor(
                f"d{k}_{l}", [P, slots], I32, kind="ExternalInput")

    h_dram = [nc.dram_tensor(f"h{l}", [NI if l == 0 else W, STRIDE], I32,
                             kind="Internal") for l in range(4)]

    with TileContext(nc) as tc:
        with (
            tc.tile_pool(name="io", bufs=1) as io_pool,
            tc.tile_pool(name="work", bufs=2) as work,
            tc.tile_pool(name="psum", bufs=1, space="PSUM") as psum_pool,
        ):
            # ---- constants in ----
            idx_t, d_t = {}, {}
            for (l, ab), t in idx_in.items():
                n = W if l < 3 else W4
                it = io_pool.tile([P, n // 16], mybir.dt.int16, tag=f"idx{ab}{l}")
                nc.sync.dma_start(out=it[:], in_=t.ap()[:])
                idx_t[(l, ab)] = it
            for (l, k), t in d_in.items():
                slots = S if l < 3 else S4
                dt_ = io_pool.tile([P, slots], I32, tag=f"d{k}_{l}")
                nc.sync.dma_start(out=dt_[:], in_=t.ap()[:])
                d_t[(l, k)] = dt_

            # ---- layer 0: threshold + bit-pack ----
            xs = io_pool.tile([P, CORES * BS], mybir.dt.float32, tag="xs")
            for t in range(CORES):
                nc.sync.dma_start(out=xs[:, t * BS:(t + 1) * BS],
                                  in_=xT.ap()[t * P:(t + 1) * P, :])
            thr = io_pool.tile([P, CORES * BS], I32, tag="thr")
            nc.vector.tensor_scalar(out=thr[:], in0=xs[:], scalar1=0.5,
                                    scalar2=None, op0=mybir.AluOpType.is_gt)
            lvl = thr
            width = CORES * BS
            for k in range(5):
                half = width // 2
                pair = lvl[:].rearrange("p (n two) -> p n two", two=2)
                tmp = work.tile([P, half], I32, tag=f"pk{k}t")
                nc.vector.tensor_scalar(out=tmp[:], in0=pair[:, :, 1],
                                        scalar1=1 << k, scalar2=None, op0=SHL)
                nxt = work.tile([P, half], I32, tag=f"pk{k}o")
                nc.vector.tensor_tensor(out=nxt[:], in0=pair[:, :, 0], in1=tmp[:],
                                        op=OR)
                lvl, width = nxt, half
            # lvl: [128, 64]; feature f=t*128+p word w at (p, t*8+w)
            nc.sync.dma_start(
                out=h_dram[0].ap()[:, :NW].rearrange("(t p) w -> p t w", p=P),
                in_=lvl[:].rearrange("p (t w) -> p t w", w=NW),
            )

            # ---- gate layers ----
            def bcast(d_tile, slots):
                a = d_tile[:]
                return bass.AP(a.tensor, a.offset, list(a.ap) + [[0, NW]])

            h4_tile = None
            for l in range(4):
                n = W if l < 3 else W4
                slots = S if l < 3 else S4
                src = h_dram[0] if l == 0 else h_dram[l]
                # NOTE: h_dram[l] holds INPUT of layer l (h_dram[0]=packed x,
                # h_dram[l]=output of layer l-1); layer 3 output stays in SBUF.
                a_t = work.tile([P, slots * NW], I32, tag="ga")
                b_t = work.tile([P, slots * NW], I32, tag="gb")
                qn = 0
                for (c0, cn) in _chunks(n):
                    s0 = c0 // P
                    sn = cn // P
                    for ab, dst in (("a", a_t), ("b", b_t)):
                        _dma_gather_small(
                            nc.gpsimd,
                            out_ap=dst[:, s0 * NW:(s0 + sn) * NW]
                                .rearrange("p (s w) -> p s w", w=NW),
                            in_ap=src.ap()[:, :NW],
                            idxs_ap=idx_t[(l, ab)][:, c0 // 16:(c0 + cn) // 16],
                            num_idxs=cn,
                            elem_size=NW,
                            elem_step=STRIDE,
                            queue_num=qn % 4,
                        )
                        qn += 1
                t_t = work.tile([P, slots * NW], I32, tag="gt")
                u_t = work.tile([P, slots * NW], I32, tag="gu")
                v3 = lambda t: t[:].rearrange("p (s w) -> p s w", w=NW)
                nc.vector.tensor_tensor(out=v3(t_t), in0=v3(a_t), in1=v3(b_t), op=AND)
                nc.vector.tensor_tensor(out=v3(t_t), in0=v3(t_t),
                                        in1=bcast(d_t[(l, 3)], slots), op=AND)
                nc.vector.tensor_tensor(out=v3(u_t), in0=v3(a_t),
                                        in1=bcast(d_t[(l, 1)], slots), op=AND)
                nc.vector.tensor_tensor(out=v3(t_t), in0=v3(t_t), in1=v3(u_t), op=XOR)
                nc.vector.tensor_tensor(out=v3(u_t), in0=v3(b_t),
                                        in1=bcast(d_t[(l, 2)], slots), op=AND)
                nc.vector.tensor_tensor(out=v3(t_t), in0=v3(t_t), in1=v3(u_t), op=XOR)
                nc.vector.tensor_tensor(out=v3(t_t), in0=v3(t_t),
                                        in1=bcast(d_t[(l, 0)], slots), op=XOR)
                if l < 3:
                    nc.sync.dma_start(
                        out=h_dram[l + 1].ap()[:, :NW]
                            .rearrange("(s p) w -> p s w", p=P),
                        in_=v3(t_t),
                    )
                else:
                    h4_tile = t_t

            # ---- GroupSum ----
            # spread nibble-phases, split to fp32-exact 16-bit halves, reduce
            # the 13 class slots (DVE reduce accumulates in fp32), widen,
            # ones-matmul over partitions.
            f_t = io_pool.tile([P, 32 * 80], mybir.dt.float32, tag="fwide")
            for k in range(4):
                sp = work.tile([P, S4 * NW], I32, tag="spread")
                if k == 0:
                    nc.vector.tensor_scalar(out=sp[:], in0=h4_tile[:],
                                            scalar1=0x11111111, scalar2=None, op0=AND)
                else:
                    nc.vector.tensor_scalar(out=sp[:], in0=h4_tile[:], scalar1=k,
                                            scalar2=0x11111111, op0=SHR, op1=AND)
                for h in range(2):
                    sph = work.tile([P, S4 * NW], I32, tag="spreadh")
                    if h == 0:
                        nc.vector.tensor_scalar(out=sph[:], in0=sp[:],
                                                scalar1=0x0000FFFF, scalar2=None,
                                                op0=AND)
                    else:
                        nc.vector.tensor_scalar(out=sph[:], in0=sp[:], scalar1=16,
                                                scalar2=None, op0=SHR)
                    rk = work.tile([P, NC * NW], I32, tag="segred")
                    sp4 = bass.AP(sph[:].tensor, sph[:].offset,
                                  [list(sph[:].ap[0]), [SLOTC * NW, NC], [1, NW],
                                   [NW, SLOTC]])
                    with nc.allow_low_precision(reason="lane sums < 2^24, exact"):
                        nc.vector.tensor_reduce(
                            out=rk[:].rearrange("p (c w) -> p c w", w=NW),
                            in_=sp4, axis=mybir.AxisListType.X,
                            op=mybir.AluOpType.add)
                    for kp in range(4):
                        e_t = work.tile([P, NC * NW], I32, tag="extr")
                        if kp == 0:
                            nc.vector.tensor_scalar(out=e_t[:], in0=rk[:],
                                                    scalar1=0xF, scalar2=None,
                                                    op0=AND)
                        else:
                            nc.vector.tensor_scalar(out=e_t[:], in0=rk[:],
                                                    scalar1=4 * kp, scalar2=0xF,
                                                    op0=SHR, op1=AND)
                        blk = (k * 4 + kp) * 2 + h
                        nc.vector.tensor_copy(
                            out=f_t[:, blk * 80:(blk + 1) * 80], in_=e_t[:])
            ones = io_pool.tile([P, 1], mybir.dt.float32, tag="ones")
            nc.vector.memset(ones[:], 1.0)
            osb = io_pool.tile([1, 16 * 160], I32, tag="osb")
            for m in range(5):
                ps = psum_pool.tile([1, 512], mybir.dt.float32, tag=f"ps{m}")
                nc.tensor.matmul(ps[:], lhsT=ones[:], rhs=f_t[:, m * 512:(m + 1) * 512])
                nc.vector.tensor_copy(out=osb[:, m * 512:(m + 1) * 512], in_=ps[:])
            # osb col layout: (k:640)(kp:160)(c:16)(w:2)(h:1);
            # sample s = 32w + 16h + 4kp + k — host reorders to [c, s].
            nc.sync.dma_start(out=out_y.ap()[:], in_=osb[:])
    nc.compile()
    return nc


_PROGRAM_CACHE = {}


def kernel(**inputs):
    x = np.asarray(inputs["x"])
    assert x.shape == (B, NI) and x.dtype == np.float32

    idx_a = [np.asarray(inputs[f"idx_a{l}"]).astype(np.int64) for l in range(4)]
    idx_b = [np.asarray(inputs[f"idx_b{l}"]).astype(np.int64) for l in range(4)]
    ops = [np.asarray(inputs[f"ops{l}"]).astype(np.int64) for l in range(4)]

    # ---- host-side constant prep (shared across cores) ----
    const_map = {}
    # layers 0..2 keep natural gate order
    for l in range(3):
        const_map[f"idxa{l}"] = _wrap_idx_chunked(idx_a[l])
        const_map[f"idxb{l}"] = _wrap_idx_chunked(idx_b[l])
        for k, m in enumerate(_anf_masks(ops[l])):
            const_map[f"d{k}_{l}"] = _tile_layout(m, S)
    # final layer: class-aligned padded relabeling
    j = np.arange(W4)
    s = j // P
    cls = s // SLOTC
    i_in = (s % SLOTC) * P + (j % P)
    real = i_in < (W // NC)
    orig = np.where(real, cls * (W // NC) + np.minimum(i_in, W // NC - 1), 0)
    ia4 = np.where(real, idx_a[3][orig], 0)
    ib4 = np.where(real, idx_b[3][orig], 0)
    const_map["idxa3"] = _wrap_idx_chunked(ia4)
    const_map["idxb3"] = _wrap_idx_chunked(ib4)
    for k, m in enumerate(_anf_masks(ops[3])):
        mm = np.where(real, m[orig], 0).astype(np.int32)
        const_map[f"d{k}_3"] = _tile_layout(mm, S4)

    key = "prog"
    if key not in _PROGRAM_CACHE:
        _PROGRAM_CACHE[key] = _build_program()
    nc = _PROGRAM_CACHE[key]

    in_maps = []
    for c in range(CORES):
        m = {"xT": np.ascontiguousarray(x[c * BS:(c + 1) * BS, :].T)}
        for l in range(4):
            m[f"idxa{l}"] = const_map[f"idxa{l}"]
            m[f"idxb{l}"] = const_map[f"idxb{l}"]
            for k in range(4):
                m[f"d{k}_{l}"] = const_map[f"d{k}_{l}"]
        in_maps.append(m)

    global _last_in_maps
    _last_in_maps = in_maps
    res = run_bass_kernel_spmd(nc, in_maps, core_ids=list(range(CORES)))

    out = np.empty((B, NC), dtype=np.int32)
    for c in range(CORES):
        raw = res.results[c]["out_y"].reshape(4, 4, 2, NC, NW)  # [k, kp, h, c, w]
        y = raw.transpose(3, 4, 2, 1, 0).reshape(NC, BS)  # s = 32w+16h+4kp+k
        out[c * BS:(c + 1) * BS, :] = y.T
    return out
